# revision 23
# baseline (speedup 1.0000x reference)
"""v12 Trainium2 kernel for nn_Augmenter: instruction-minimal batched design.

Empirical finding on this setup: per-repeat cost is dominated by a fixed
per-INSTRUCTION overhead (~40-120us each; DVE ~60us, ACT ~120us), not by
data movement (the gather baseline's 5.6ms ~= its 128 instructions x 44us).
So this kernel minimizes instructions: ~13 per repeat, all batched over the
16 images.

Layout: partition p = 8g+q holds 40 consecutive "spans" (rows) of image g's
translated window; the whole translation (row AND col shift, with zero fill
from padding) is folded into 128 per-partition flat element offsets consumed
by ONE gpsimd indirect DMA:

  offset[8g+q] = base_g + txs*960 + tys*3 + q*38400

Each span is 960 contiguous elements = one padded row [320col x 3c]; the
output pixel (r', c', ch) of image g sits at span l = r'+32 (l in [32,288)),
element m = 96 + 3c' + ch.  Sum over the full loaded window = exact image
sum S (each pixel appears exactly once; the rest is guard/pad zeros).

Math per pixel: out = M*(A*x + Bp*MC + D) with M = validity*cutout mask,
A/Bp/M/MA=A*M/BpM=Bp*M host-precomputed (param-only, like the baseline's
host-precomputed index tables), D = bb + dmp*S computed on device.

Per-repeat instruction list:
  1 DVE  OFFV = OFFBASE + SH            (re-derive offsets, i32)
  1 Pool indirect load XF [128, 40*960]
  2 DVE  MC  = x0+x1 ; += x2            ([128,40,256] stride-3 slices)
  1 DVE  red = reduce(XF) -> [128,1]    (f32)
  1 PE   psS16[16,1] = E8^T @ red       (per-image sums)
  1 DVE  d16 = psS16*dmpP + bbP         (per-partition scalar APs)
  1 PE   psD[128,1] = E8T^T @ d16       (broadcast D back to (g,q) layout)
  1 DVE  GM  = MC * BpM                 (fp16)
  1 DVE  GM  = M*psD + GM               (scalar_tensor_tensor, D folded)
  1 DVE  XFc = XFc * MA                 (4D TT, in-place on center)
  1 DVE  XFc = XFc + GM (bcast ch)      (4D TT)
  1 DMA  store XF center -> out
"""

import numpy as np

import concourse.bacc as bacc
import concourse.bass as bass
import concourse.mybir as mybir
import concourse.tile as tile

F32 = mybir.dt.float32
F16 = mybir.dt.float16
I32 = mybir.dt.int32
OP = mybir.AluOpType
AX = mybir.AxisListType

N_CORES = 8
B_FULL = 128
N = B_FULL // N_CORES  # 16 images per core
C, H, W = 3, 256, 256
EW = 960            # elems per padded row (320 cols x 3c)
GF, GB = 36, 40     # guard rows front/back
TOTR = GF + N * 320 + GB
SPANS = 40          # spans per partition (8 partitions per image)
CHUNK = SPANS * EW  # 38400 elems per partition


def build_kernel(repeat: int = 1, mode: str = "full"):
    nc = bacc.Bacc(
        "TRN2",
        target_bir_lowering=False,
        debug=False,
        enable_asserts=False,
        num_devices=N_CORES,
    )
    imgp_t = nc.dram_tensor("imgp", [TOTR * EW, 1], F16, kind="ExternalInput")
    msk_t = nc.dram_tensor("msk", [128, 3 * SPANS * 256], F16, kind="ExternalInput")
    off_t = nc.dram_tensor("off", [128, 3], I32, kind="ExternalInput")
    e8_t = nc.dram_tensor("e8", [128, N], F32, kind="ExternalInput")
    e8t_t = nc.dram_tensor("e8t", [N, 128], F32, kind="ExternalInput")
    db_t = nc.dram_tensor("db", [N, 2], F32, kind="ExternalInput")
    out_t = nc.dram_tensor("out", [128, SPANS * 768], F16, kind="ExternalOutput")
    imgp = imgp_t.ap()

    with tile.TileContext(nc) as tc:
        with (
            tc.tile_pool(name="cst", bufs=1) as cpool,
            tc.tile_pool(name="xf", bufs=1) as xfpool,
            tc.tile_pool(name="mc", bufs=1) as mcpool,
            tc.tile_pool(name="sm", bufs=2) as smpool,
            tc.tile_pool(name="ps", bufs=2, space="PSUM") as pspool,
        ):
            V = nc.vector

            # ---- setup ----
            MM3 = cpool.tile([128, 3, SPANS * 256], F16)
            nc.sync.dma_start(MM3, msk_t.ap().rearrange("p (k f) -> p k f", k=3))
            M_ = MM3[:, 0, :]
            MA = MM3[:, 1, :]
            BPM = MM3[:, 2, :]

            OFF3 = cpool.tile([128, 3], I32)
            nc.sync.dma_start(OFF3, off_t.ap())
            OFFBASE = OFF3[:, 0:2]  # two half-chunk bases per partition
            SH = OFF3[:, 2:3]

            E8 = cpool.tile([128, N], F32)
            nc.sync.dma_start(E8, e8_t.ap())
            E8T = cpool.tile([N, 128], F32)
            nc.sync.dma_start(E8T, e8t_t.ap())
            DB = cpool.tile([N, 2], F32)
            nc.sync.dma_start(DB, db_t.ap())
            DMPP = DB[:, 0:1]
            BBP = DB[:, 1:2]

            # ---- repeat loop ----
            for rep in range(repeat):
                offv = smpool.tile([128, 2], I32, tag="offv")
                V.tensor_tensor(offv, OFFBASE,
                                SH.broadcast_to([128, 2]), OP.add)

                xf = xfpool.tile([128, SPANS, EW], F16, tag="xf")
                half = SPANS // 2
                for hv in range(2):
                    nc.gpsimd.indirect_dma_start(
                        out=xf[:, hv * half : (hv + 1) * half, :].rearrange(
                            "p s e -> p (s e)"),
                        out_offset=None,
                        in_=imgp,
                        in_offset=bass.IndirectOffsetOnAxis(
                            ap=offv[:, hv : hv + 1], axis=0),
                    )

                # center view: [128, SPANS, 256, 3] at elem offset 96
                xc4 = xf[:, :, 96:864].rearrange("p s (w c) -> p s w c", c=3)
                x0 = xc4[:, :, :, 0]
                x1 = xc4[:, :, :, 1]
                x2 = xc4[:, :, :, 2]

                mc = mcpool.tile([128, SPANS, 256], F16, tag="mc")
                V.tensor_tensor(mc, x0, x1, OP.add)
                V.tensor_tensor(mc, mc, x2, OP.add)

                red = smpool.tile([128, 1], F32, tag="red")
                V.tensor_reduce(red, xf.rearrange("p s e -> p (s e)"),
                                AX.X, OP.add)

                psS = pspool.tile([N, 1], F32, tag="psS")
                nc.tensor.matmul(psS, lhsT=E8, rhs=red, start=True, stop=True)
                d16 = smpool.tile([N, 1], F32, tag="d16")
                V.tensor_scalar(d16, psS, DMPP, BBP, OP.mult, OP.add)
                psD = pspool.tile([128, 1], F32, tag="psD")
                nc.tensor.matmul(psD, lhsT=E8T, rhs=d16, start=True, stop=True)

                mv = M_.rearrange("p (s w) -> p s w", s=SPANS)
                mav = MA.rearrange("p (s w) -> p s w", s=SPANS)
                bpmv = BPM.rearrange("p (s w) -> p s w", s=SPANS)

                gm = mcpool.tile([128, SPANS, 256], F16, tag="gm")
                V.tensor_tensor(gm, mc, bpmv, OP.mult)
                # gm = M*D + gm   (D per-partition scalar from PSUM)
                V.scalar_tensor_tensor(
                    out=gm, in0=mv, scalar=psD[:, 0:1], in1=gm,
                    op0=OP.mult, op1=OP.add)

                # out = x*MA + gm  (in place on xf center)
                V.tensor_tensor(
                    xc4, xc4,
                    mav.unsqueeze(3).broadcast_to([128, SPANS, 256, 3]),
                    OP.mult)
                V.tensor_tensor(
                    xc4, xc4,
                    gm.unsqueeze(3).broadcast_to([128, SPANS, 256, 3]),
                    OP.add)

                nc.scalar.dma_start(
                    out_t.ap().rearrange("p (s e) -> p s e", s=SPANS),
                    xf[:, :, 96:864],
                )

    nc.compile()
    return nc


# ---------------------------------------------------------------------------
# Host wrapper
# ---------------------------------------------------------------------------

from concourse.bass_utils import run_bass_kernel_spmd

_CACHE = {}


def _get_compiled(repeat):
    if repeat not in _CACHE:
        _CACHE[repeat] = build_kernel(repeat)
    return _CACHE[repeat]


def _pack_core(imgs, br, sat, con, tx, ty, cx, cy):
    n = imgs.shape[0]
    buf = np.zeros((TOTR, EW), np.float16)
    # buf[GF + g*320 + 32 + r, (32+col)*3 + c] = imgs[g, c, r, col]
    b4 = buf.reshape(TOTR, 320, 3)
    for g in range(n):
        r0 = GF + g * 320 + 32
        b4[r0 : r0 + H, 32 : 32 + W, :] = imgs[g].transpose(1, 2, 0)
    imgp = buf.reshape(TOTR * EW, 1)

    br = br.reshape(n).astype(np.float64)
    sat = sat.reshape(n).astype(np.float64)
    con = con.reshape(n).astype(np.float64)
    txs = tx.reshape(n).astype(np.int64) - 32
    tys = ty.reshape(n).astype(np.int64) - 32
    cxv = cx.reshape(n).astype(np.int64)
    cyv = cy.reshape(n).astype(np.int64)

    cf = con + 0.5
    s2 = 2.0 * sat
    A = cf * s2
    Bp = cf * (1.0 - s2) / 3.0
    bb = br - 0.5
    dmp = (1.0 - cf) / (C * H * W)

    # masks in (p=8g+q, span j, col c') layout; span l = 40q+j, r' = l-32
    g_of_p = np.arange(128) // 8
    q_of_p = np.arange(128) % 8
    l_idx = q_of_p[:, None] * SPANS + np.arange(SPANS)[None, :]  # [128, 40]
    rprime = l_idx - 32  # output row, may be out of [0,256)
    cprime = np.arange(256)[None, None, :]  # [1, 1, 256]

    txp = txs[g_of_p][:, None, None]
    typ = tys[g_of_p][:, None, None]
    rp = rprime[:, :, None]
    inframe = (rp >= 0) & (rp < 256)
    rv = (rp + txp >= 0) & (rp + txp <= 255)
    cv = (cprime + typ >= 0) & (cprime + typ <= 255)
    lox = np.maximum(cxv - 64, 0)[g_of_p][:, None, None]
    hix = np.minimum(cxv + 63, 255)[g_of_p][:, None, None]
    loy = np.maximum(cyv - 64, 0)[g_of_p][:, None, None]
    hiy = np.minimum(cyv + 63, 255)[g_of_p][:, None, None]
    rin = (rp >= lox) & (rp <= hix)
    cin = (cprime >= loy) & (cprime <= hiy)
    M = (inframe & rv & cv & ~(rin & cin)).astype(np.float64)  # [128,40,256]

    msk = np.empty((128, 3, SPANS * 256), np.float16)
    msk[:, 0] = M.reshape(128, -1)
    msk[:, 1] = (A[g_of_p][:, None, None] * M).reshape(128, -1)
    msk[:, 2] = (Bp[g_of_p][:, None, None] * M).reshape(128, -1)

    base_g = (GF + g_of_p * 320) * EW
    off = np.empty((128, 3), np.int32)
    off[:, 0] = base_g + q_of_p * CHUNK
    off[:, 1] = base_g + q_of_p * CHUNK + (SPANS // 2) * EW
    off[:, 2] = (txs[g_of_p] * EW + tys[g_of_p] * 3).astype(np.int32)

    e8 = (np.arange(128)[:, None] // 8 == np.arange(n)[None, :]).astype(np.float32)
    e8t = (np.arange(128)[None, :] // 8 == np.arange(n)[:, None]).astype(np.float32)
    db = np.stack([dmp, bb], axis=1).astype(np.float32)

    return {
        "imgp": imgp,
        "msk": msk.reshape(128, 3 * SPANS * 256),
        "off": off,
        "e8": e8,
        "e8t": e8t,
        "db": db,
    }


def kernel(imgs, br, sat, con, tx, ty, cx, cy, _repeat=1):
    imgs = np.asarray(imgs, np.float32)
    br = np.asarray(br, np.float32)
    sat = np.asarray(sat, np.float32)
    con = np.asarray(con, np.float32)
    tx = np.asarray(tx, np.int32)
    ty = np.asarray(ty, np.int32)
    cx = np.asarray(cx, np.int32)
    cy = np.asarray(cy, np.int32)

    nc = _get_compiled(_repeat)
    in_maps = []
    for k in range(N_CORES):
        sl = slice(k * N, (k + 1) * N)
        in_maps.append(
            _pack_core(
                imgs[sl], br[sl], sat[sl], con[sl], tx[sl], ty[sl], cx[sl], cy[sl]
            )
        )
    res = run_bass_kernel_spmd(nc, in_maps, core_ids=list(range(N_CORES)))
    outf = np.empty((B_FULL, C, H, W), np.float32)
    for k in range(N_CORES):
        o = np.asarray(res.results[k]["out"]).reshape(128, SPANS, 256, 3)
        # o[8g+q, j, c', ch] -> img[g, ch, r'=40q+j-32, c']
        o5 = o.reshape(N, 8, SPANS, 256, 3)  # [g, q, j, c', ch]
        rows = o5.transpose(0, 4, 1, 2, 3).reshape(N, 3, 8 * SPANS, 256)
        outf[k * N : (k + 1) * N] = rows[:, :, 32 : 32 + H, :].astype(np.float32)
    return outf
